# revision 12
# baseline (speedup 1.0000x reference)
"""Cross linear-attention (2-branch) Trainium2 kernel.

Sharding: spatial over image rows. 8 cores x 16 rows each (1-row halo).
Each core handles both batches and both branches. One tiny AllReduce
for the global attention statistics (attn/ksum/vsum per (b,branch)).

conv1x1 + depthwise3x3 are fused into 9 accumulating matmuls per
output channel group with host-precomputed tap weights
K2[tap][c_in, o] = W[o, c_in] * w_dw[o, tap].
"""
import sys
import numpy as np

sys.path.insert(0, "/opt/trn_rl_repo")

import concourse.bass as bass
import concourse.mybir as mybir
import concourse.bacc as bacc
import concourse.tile as tile
from concourse import bass_utils

DT = mybir.dt
F32 = DT.float32
F32R = DT.float32r
BF16 = DT.bfloat16

C = 128
HEADS = 8
CP = 16
H = 128
W = 128
B = 2
NCORES = 8
ROWS = H // NCORES          # 16 output rows per core
HROWS = ROWS + 2            # with halo
NL = ROWS * W               # 2048 local positions
WP = W + 2                  # padded row width 130
NG = H * W                  # 16384 global positions
EPS = 1e-6
NEPS = float(NG) + EPS

_CACHE = {}


def _build_nc():
    nc = bacc.Bacc("TRN2", target_bir_lowering=False, debug=False,
                   num_devices=NCORES)

    x_d = nc.dram_tensor("x", [B, 2, C, HROWS, W], F32R, kind="ExternalInput")
    k2_d = nc.dram_tensor("k2w", [C, 2 * 9 * 3 * C], F32R, kind="ExternalInput")
    pj_d = nc.dram_tensor("projw", [C, 2 * C], BF16, kind="ExternalInput")
    ee_d = nc.dram_tensor("ee", [C, C], BF16, kind="ExternalInput")
    tp_d = nc.dram_tensor("tempc", [C, 2], F32, kind="ExternalInput")
    mk_d = nc.dram_tensor("msk", [C, 32], F32, kind="ExternalInput")
    id_d = nc.dram_tensor("ident", [C, C], BF16, kind="ExternalInput")
    out_d = nc.dram_tensor("out", [B, 2, C, ROWS, W], F32,
                           kind="ExternalOutput")
    dbg_d = nc.dram_tensor("dbg", [C, 520 + 3 * C + 2048], F32,
                           kind="ExternalOutput")

    with tile.TileContext(nc) as tc:
        with (
            tc.tile_pool(name="wpool", bufs=1) as wpool,
            tc.tile_pool(name="xpool", bufs=2) as xpool,
            tc.tile_pool(name="qkv", bufs=2) as qkvp,
            tc.tile_pool(name="qlive", bufs=4) as qlive,
            tc.tile_pool(name="tp", bufs=2) as tpp,
            tc.tile_pool(name="tail", bufs=1) as tailp,
            tc.tile_pool(name="stat", bufs=1) as statp,
            tc.tile_pool(name="psc", bufs=2, space="PSUM") as psc,
            tc.tile_pool(name="psa", bufs=1, space="PSUM") as psa,
            tc.tile_pool(name="pst", bufs=1, space="PSUM") as pst,
            tc.tile_pool(name="pss", bufs=1, space="PSUM") as pss,
            tc.tile_pool(name="dram", bufs=1, space="DRAM") as dramp,
        ):
            # ---- static weights ----
            k2_sb = wpool.tile([C, 2 * 9 * 3 * C], F32R)
            nc.sync.dma_start(k2_sb[:], k2_d.ap())
            pj_sb = wpool.tile([C, 2 * C], BF16)
            nc.sync.dma_start(pj_sb[:], pj_d.ap())
            ee_sb = wpool.tile([C, C], BF16)
            nc.sync.dma_start(ee_sb[:], ee_d.ap())
            tp_sb = wpool.tile([C, 2], F32)
            nc.sync.dma_start(tp_sb[:], tp_d.ap())
            mk_sb = wpool.tile([C, 32], F32)
            nc.sync.dma_start(mk_sb[:], mk_d.ap())
            id_sb = wpool.tile([C, C], BF16)
            nc.sync.dma_start(id_sb[:], id_d.ap())

            stats_sb = statp.tile([C, 4 * 130], F32)
            stats_rd = statp.tile([C, 4 * 130], F32)

            units = [(b, br) for b in range(B) for br in range(2)]

            # per-unit saved tiles for the tail phase
            q_sbs, qn_parts, dbg_tiles = [], [], []

            for u, (b, br) in enumerate(units):
                # ---- load input slice (zero-padded cols) ----
                x_pad = xpool.tile([C, HROWS, WP], F32R, tag="xpad")
                nc.vector.memset(x_pad[:, :, 0:1].bitcast(F32), 0.0)
                nc.vector.memset(x_pad[:, :, W + 1:W + 2].bitcast(F32), 0.0)
                nc.sync.dma_start(x_pad[:, :, 1:W + 1], x_d.ap()[b, br])

                # ---- fused conv3x3 (qkv) ----
                # groups g: 0=q, 1=k, 2=v ; psum [C, 1024] per (g, half)
                q_sb = qlive.tile([C, NL], BF16, tag="q")
                k_sb = qkvp.tile([C, NL], BF16, tag="k")
                v_sb = qkvp.tile([C, NL], BF16, tag="v")
                vsum2 = tpp.tile([C, 2], F32, tag="vs2")
                g_dst = [q_sb, k_sb, v_sb]

                for hh in range(2):          # column halves (8 rows each)
                    for g in range(3):
                        ps = psc.tile([C, 1024], F32, tag="conv")
                        for t in range(9):
                            dy, dx = t // 3, t % 3
                            wslice = k2_sb[:, ((br * 9 + t) * 3 + g) * C:
                                           ((br * 9 + t) * 3 + g + 1) * C]
                            for cc in range(2):
                                r0 = hh * 8 + cc * 4
                                rhs = x_pad[:, r0 + dy:r0 + dy + 4,
                                            dx:dx + W]
                                nc.tensor.matmul(
                                    ps[:, cc * 512:(cc + 1) * 512],
                                    wslice, rhs,
                                    start=(t == 0), stop=(t == 8))
                        # evict: q,v on ACT (v with accum for vsum), k on DVE
                        dst = g_dst[g][:, hh * 1024:(hh + 1) * 1024]
                        if g == 0:
                            nc.scalar.copy(dst, ps[:])
                        elif g == 1:
                            nc.vector.tensor_copy(dst, ps[:])
                        else:
                            nc.scalar.activation(
                                dst, ps[:],
                                mybir.ActivationFunctionType.Copy,
                                accum_out=vsum2[:, hh:hh + 1])

                # ---- transposes (bf16 DMA transpose, 16 chunks each) ----
                kT = tpp.tile([C, CP, C], BF16, tag="kT")
                vhT = tpp.tile([C, CP, C + 1], BF16, tag="vhT")
                nc.vector.memset(vhT[:, :, C:C + 1], 1.0)
                for src_sb, dstT, dsl in ((k_sb, kT, None), (v_sb, vhT, C)):
                    for c4 in range(4):
                        pt = pst.tile([C, 512], BF16, tag="tp")
                        for j in range(4):
                            ch = c4 * 4 + j
                            nc.tensor.transpose(
                                pt[:, j * C:(j + 1) * C],
                                src_sb[:, ch * C:(ch + 1) * C], id_sb[:])
                        if dsl is None:
                            nc.scalar.copy(
                                dstT[:, c4 * 4:(c4 + 1) * 4, :], pt[:])
                        else:
                            nc.scalar.copy(
                                dstT[:, c4 * 4:(c4 + 1) * 4, 0:C],
                                pt[:].rearrange("p (a b) -> p a b", a=4))

                # ---- kn^2 -> invkn  (post-transpose layout [n, (ch,h,cp)])
                ksq = tpp.tile([C, NL], BF16, tag="ksq")
                nc.vector.tensor_mul(ksq[:], kT[:, :, :], kT[:, :, :])
                kn2 = tpp.tile([C, CP, HEADS], F32, tag="kn2")
                nc.vector.reduce_sum(
                    kn2[:],
                    ksq[:].rearrange("p (c h d) -> p (c h) d", c=CP, h=HEADS,
                                     d=CP),
                    axis=mybir.AxisListType.X)
                kn = tpp.tile([C, CP, HEADS], F32, tag="kn")
                nc.scalar.sqrt(kn[:], kn2[:])
                ikn = tpp.tile([C, CP, HEADS], F32, tag="ikn")
                nc.vector.reciprocal_approx_fast(ikn[:], kn[:])
                iknb = tpp.tile([C, CP, HEADS], BF16, tag="iknb")
                nc.vector.tensor_copy(iknb[:], ikn[:])

                # k^ = kT * invkn  (broadcast over cp within head)
                khT = tpp.tile([C, CP, C], BF16, tag="khT")
                for ch in range(CP):
                    nc.vector.tensor_mul(
                        khT[:, ch, :].rearrange("p (h d) -> p h d", h=HEADS),
                        kT[:, ch, :].rearrange("p (h d) -> p h d", h=HEADS),
                        iknb[:, ch, :].broadcast_to([C, HEADS, CP]))

                if u == 0:
                    dbg_tiles.extend([kT, khT])
                # ---- local attn stats: [attn | ksum] ----
                ps_at = psa.tile([C, 129], F32, tag="attn")
                for ch in range(CP):
                    nc.tensor.matmul(ps_at[:], khT[:, ch, :], vhT[:, ch, :],
                                     start=(ch == 0), stop=(ch == CP - 1))
                nc.scalar.copy(stats_sb[:, u * 130:u * 130 + 129], ps_at[:])
                nc.vector.tensor_add(stats_sb[:, u * 130 + 129:u * 130 + 130],
                                     vsum2[:, 0:1], vsum2[:, 1:2])

                # ---- qn^2 via EE matmul needs q^2 ----
                q2 = tpp.tile([C, NL], BF16, tag="q2")
                nc.vector.tensor_mul(q2[:], q_sb[:], q_sb[:])
                qn = qlive.tile([C, NL], F32, tag="qn")
                for hh in range(2):
                    ps = pss.tile([C, 1024], F32, tag="small")
                    for cc in range(2):
                        nc.tensor.matmul(ps[:, cc * 512:(cc + 1) * 512],
                                         ee_sb[:],
                                         q2[:, hh * 1024 + cc * 512:
                                            hh * 1024 + (cc + 1) * 512],
                                         start=True, stop=True)
                    nc.scalar.sqrt(qn[:, hh * 1024:(hh + 1) * 1024], ps[:])
                q_sbs.append(q_sb)
                qn_parts.append(qn)

            # ---- AllReduce the stats ----
            d_in = dramp.tile([C, 4 * 130], F32)
            d_out = dramp.tile([C, 4 * 130], F32)
            nc.gpsimd.dma_start(d_in[:], stats_sb[:])
            nc.gpsimd.collective_compute(
                "AllReduce", mybir.AluOpType.add,
                replica_groups=[list(range(NCORES))],
                ins=[d_in.opt()], outs=[d_out.opt()])
            nc.sync.dma_start(stats_rd[:], d_out[:])

            dbg_sb = statp.tile([C, 520 + 3 * C + 2048], F32)
            nc.vector.tensor_copy(dbg_sb[:, 0:520], stats_rd[:])
            nc.vector.tensor_copy(dbg_sb[:, 520:520 + C],
                                  dbg_tiles[0][:, 0, :])
            nc.vector.tensor_copy(dbg_sb[:, 520 + C:520 + 2 * C],
                                  q_sbs[0][:, 0:C])
            nc.vector.tensor_copy(dbg_sb[:, 520 + 2 * C:520 + 3 * C],
                                  dbg_tiles[1][:, 0, :])
            nc.vector.tensor_copy(dbg_sb[:, 520 + 3 * C:],
                                  qn_parts[0][:])
            nc.sync.dma_start(dbg_d.ap(), dbg_sb[:])

            # ---- tail per unit: P,D mms + num/den + proj ----
            for u, (b, br) in enumerate(units):
                # cross-attention: use stats of the OTHER branch, same batch
                uo = (u // 2) * 2 + (1 - br)
                uob = uo * 130
                q_sb, qn = q_sbs[u], qn_parts[u]

                lhP = tailp.tile([C, C], BF16, tag="lhP")
                lhD = tailp.tile([C, C], BF16, tag="lhD")
                nc.vector.memset(lhP[:], 0.0)
                nc.vector.memset(lhD[:], 0.0)
                for g in range(4):
                    sp = slice(32 * g, 32 * (g + 1))
                    nc.vector.tensor_mul(
                        lhP[sp, 32 * g:32 * (g + 1)],
                        stats_rd[sp, uob + 32 * g:uob + 32 * (g + 1)],
                        mk_sb[sp, :])
                    nc.vector.tensor_scalar_mul(
                        lhD[sp, 32 * g:32 * (g + 1)],
                        mk_sb[sp, :],
                        stats_rd[sp, uob + 128:uob + 129])
                vsumR = stats_rd[:, uob + 129:uob + 130]

                nume = tailp.tile([C, NL], F32, tag="nume")
                deni = tailp.tile([C, NL], F32, tag="deni")
                recd = tailp.tile([C, NL], F32, tag="recd")
                outp = tailp.tile([C, NL], BF16, tag="outp")
                out_sb = tailp.tile([C, NL], F32, tag="outsb")

                for hh in range(2):
                    sl = slice(hh * 1024, (hh + 1) * 1024)
                    psP = pss.tile([C, 1024], F32, tag="small")
                    for cc in range(2):
                        s2 = slice(hh * 1024 + cc * 512,
                                   hh * 1024 + (cc + 1) * 512)
                        nc.tensor.matmul(psP[:, cc * 512:(cc + 1) * 512],
                                         lhP[:], q_sb[:, s2],
                                         start=True, stop=True)
                    nc.vector.scalar_tensor_tensor(
                        nume[:, sl], qn[:, sl], vsumR, psP[:],
                        op0=mybir.AluOpType.mult, op1=mybir.AluOpType.add)
                    psD = pss.tile([C, 1024], F32, tag="small")
                    for cc in range(2):
                        s2 = slice(hh * 1024 + cc * 512,
                                   hh * 1024 + (cc + 1) * 512)
                        nc.tensor.matmul(psD[:, cc * 512:(cc + 1) * 512],
                                         lhD[:], q_sb[:, s2],
                                         start=True, stop=True)
                    nc.vector.scalar_tensor_tensor(
                        deni[:, sl], qn[:, sl], NEPS, psD[:],
                        op0=mybir.AluOpType.mult, op1=mybir.AluOpType.add)

                nc.vector.reciprocal_approx_fast(recd[:], deni[:])
                nc.vector.scalar_tensor_tensor(
                    outp[:], nume[:], tp_sb[:, br:br + 1], recd[:],
                    op0=mybir.AluOpType.mult, op1=mybir.AluOpType.mult)

                for hh in range(2):
                    psO = pss.tile([C, 1024], F32, tag="small")
                    for cc in range(2):
                        s2 = slice(hh * 1024 + cc * 512,
                                   hh * 1024 + (cc + 1) * 512)
                        nc.tensor.matmul(
                            psO[:, cc * 512:(cc + 1) * 512],
                            pj_sb[:, br * C:(br + 1) * C],
                            outp[:, s2],
                            start=True, stop=True)
                    nc.scalar.copy(out_sb[:, hh * 1024:(hh + 1) * 1024],
                                   psO[:])
                nc.sync.dma_start(out_d.ap()[b, br], out_sb[:])

    nc.compile()
    return nc


def _prep_inputs(feat, qkv1_w, dw1_w, proj1_w, qkv2_w, dw2_w, proj2_w,
                 temp1, temp2):
    feat = np.asarray(feat, dtype=np.float32)
    # fused conv weights: K2[br, t, g][c_in, o] = W[o, c_in] * dw[o, t]
    k2 = np.zeros((2, 9, 3, C, C), np.float32)
    for br, (qw, dw) in enumerate([(qkv1_w, dw1_w), (qkv2_w, dw2_w)]):
        Wm = np.asarray(qw, np.float32)[:, :, 0, 0]          # [384, 128]
        Dm = np.asarray(dw, np.float32)[:, 0].reshape(3 * C, 9)
        for t in range(9):
            scaled = Wm * Dm[:, t:t + 1]                      # [384, 128]
            for g in range(3):
                k2[br, t, g] = scaled[g * C:(g + 1) * C].T    # [c_in, o]
    k2_host = k2.transpose(3, 0, 1, 2, 4).reshape(C, -1).copy()

    import ml_dtypes
    pj = np.stack([np.asarray(proj1_w, np.float32)[:, :, 0, 0].T,
                   np.asarray(proj2_w, np.float32)[:, :, 0, 0].T],
                  axis=0).transpose(1, 0, 2).reshape(C, 2 * C).copy()
    pj = pj.astype(ml_dtypes.bfloat16)

    ee = np.zeros((C, C), np.float32)
    for h in range(HEADS):
        ee[h * CP:(h + 1) * CP, h * CP:(h + 1) * CP] = 1.0
    ee = ee.astype(ml_dtypes.bfloat16)

    msk = np.zeros((C, 32), np.float32)
    for p in range(C):
        msk[p, (p % 32) // 16 * 16:(p % 32) // 16 * 16 + 16] = 1.0

    idm = np.eye(C, dtype=np.float32).astype(ml_dtypes.bfloat16)

    tpc = np.zeros((C, 2), np.float32)
    tpc[:, 0] = np.repeat(np.asarray(temp1, np.float32).ravel(), CP)
    tpc[:, 1] = np.repeat(np.asarray(temp2, np.float32).ravel(), CP)

    fpad = np.zeros((B, 2 * C, H + 2, W), np.float32)
    fpad[:, :, 1:H + 1] = feat

    in_maps = []
    for ci in range(NCORES):
        sl = fpad[:, :, ci * ROWS:ci * ROWS + HROWS]          # [2,256,18,128]
        x = sl.reshape(B, 2, C, HROWS, W).copy()
        in_maps.append({"x": x, "k2w": k2_host, "projw": pj, "ee": ee,
                        "tempc": tpc, "msk": msk, "ident": idm})
    return in_maps


def _run(in_maps, trace=False):
    if "nc" not in _CACHE:
        _CACHE["nc"] = _build_nc()
    nc = _CACHE["nc"]
    res = bass_utils.run_bass_kernel_spmd(
        nc, in_maps, core_ids=list(range(NCORES)), trace=trace)
    return res


def kernel(feat, qkv1_w, dw1_w, proj1_w, qkv2_w, dw2_w, proj2_w,
           temp1, temp2, _trace=False, _ret_res=False):
    in_maps = _prep_inputs(feat, qkv1_w, dw1_w, proj1_w, qkv2_w, dw2_w,
                           proj2_w, temp1, temp2)
    res = _run(in_maps, trace=_trace)
    out = np.zeros((B, 2 * C, H, W), np.float32)
    for ci in range(NCORES):
        o = res.results[ci]["out"]                            # [2,2,128,16,128]
        for br in range(2):
            out[:, br * C:(br + 1) * C, ci * ROWS:(ci + 1) * ROWS] = o[:, br]
    if _ret_res:
        return out, res
    return out
